# revision 2
# baseline (speedup 1.0000x reference)
"""GATConv model (2xGAT + GCN + Set2Set + MLP) on 8 TRN2 cores.

Host: graph-aligned node sharding (8 graphs/core), per-128-dst-node tiles,
dst-sorted edges bucketed by src gid (int16 dma_gather), all BN folded.
Device: per-tile record gather + one-hot matmul aggregation; AllGather of
node-record tables between layers; Set2Set data-parallel over graphs.
"""
import numpy as np
import ml_dtypes

import concourse.bass as bass
import concourse.bacc as bacc
import concourse.mybir as mybir
import concourse.tile as tile
from concourse._compat import cdiv, with_exitstack
from concourse.library_config import mlp as MLP_LIB
from contextlib import ExitStack

P = 128
M = 8           # cores
H = 4           # heads
CH = 64         # per-head channels
HC = H * CH     # 256
CG = 64         # gcn/hidden dim
COUT = 10
LSTM_STEPS = 10
BN_EPS = 1e-5
ANEG = -400.0   # pad-edge attention logit
SMASK = -30000.0  # set2set pad-node mask

bf16 = mybir.dt.bfloat16
f32 = mybir.dt.float32
i16 = mybir.dt.int16
nbf = ml_dtypes.bfloat16


# ---------------------------------------------------------------- host side
def preprocess(x, edge_index, batch_idx, edge_attr, params, bucket=32768,
               nwin=None):
    x = np.asarray(x, np.float32)
    ei = np.asarray(edge_index, np.int64)
    b = np.asarray(batch_idx, np.int64)
    ea = np.asarray(edge_attr, np.float32)
    N = x.shape[0]
    E = ei.shape[1]
    B = int(b.max()) + 1 if b.size else 1
    assert B % M == 0, B
    GPC = B // M

    cnt = np.bincount(b, minlength=B)
    TG = max(1, int(np.max((cnt + P - 1) // P)))
    T = GPC * TG
    NPAD = P * T
    gstart = np.zeros(B + 1, np.int64)
    np.cumsum(cnt, out=gstart[1:])

    # node -> padded gid
    core_of_g = np.arange(B) // GPC
    slot_of_g = np.arange(B) % GPC
    within = np.arange(N) - gstart[b]
    lid = slot_of_g[b] * TG * P + within
    gid = core_of_g[b] * NPAD + lid
    assert gid.max() < M * NPAD

    # full edge list with self loops
    src = np.concatenate([ei[0], np.arange(N)])
    dst = np.concatenate([ei[1], np.arange(N)])
    ew = np.concatenate([ea[:, 0], np.ones(N, np.float32)])
    ea_f = np.concatenate([ea[:, 0], np.full(N, ea.mean(), np.float32)])
    gsrc = gid[src]
    gdst = gid[dst]
    ET = src.shape[0]

    # attention edge terms: a_edge[e,h] = ea_f[e] * c[h]
    def aedge(p):
        c = (np.asarray(p['lin_e'], np.float32).reshape(H, CH)
             * np.asarray(p['att_edge'], np.float32)).sum(-1)
        return ea_f[:, None] * c[None, :]
    ae1 = aedge(params['gat1'])
    ae2 = aedge(params['gat2'])

    # gcn norm (host): deg over dst with ww
    deg = np.zeros(N, np.float64)
    np.add.at(deg, dst, ew.astype(np.float64))
    dis = np.where(deg > 0, deg ** -0.5, 0.0).astype(np.float32)
    norm = dis[src] * ew * dis[dst]

    # edge -> (core, tile, bucket)
    ecore = gdst // NPAD
    etile = (gdst % NPAD) // P
    edloc = (gdst % NPAD) % P
    R = M * NPAD
    is_self = np.arange(ET) >= E
    nbuck = max(1, cdiv(R, bucket)) if nwin is None else nwin
    if nbuck == 1:
        bases = np.array([0], np.int64)
        ebuck = np.zeros(ET, np.int64)
    else:
        # overlapping windows of width `bucket`; edges in overlaps assigned
        # to the lighter window per (core, tile) to flatten the max budget
        step = (R - bucket) / (nbuck - 1)
        bases = np.round(np.arange(nbuck) * step).astype(np.int64)
        bases[-1] = R - bucket
        wmax = np.searchsorted(bases, gsrc, side='right') - 1
        wmin = np.searchsorted(bases + bucket, gsrc, side='right')
        key_ct = ecore * T + etile
        ebuck = wmax.copy()
        dual = wmin < wmax
        # greedy: count single-eligible loads, then flip dual edges toward
        # the lighter of the two windows, processed per (core,tile)
        loads = np.zeros((M * T, nbuck), np.int64)
        np.add.at(loads, (key_ct[~dual & ~is_self], wmax[~dual & ~is_self]), 1)
        dd = np.flatnonzero(dual & ~is_self)
        ddk = key_ct[dd]
        ordd = np.argsort(ddk, kind='stable')
        dd = dd[ordd]
        for e in dd:
            k = key_ct[e]
            a, b_ = wmin[e], wmax[e]
            w = a if loads[k, a] <= loads[k, b_] else b_
            ebuck[e] = w
            loads[k, w] += 1

    # per-bucket budgets (uniform across cores+tiles); self-loops excluded
    counts = np.zeros((M, T, nbuck), np.int64)
    np.add.at(counts, (ecore[~is_self], etile[~is_self], ebuck[~is_self]), 1)
    NB = [max(P, int(cdiv(int(counts[:, :, bb].max()), P) * P))
          for bb in range(nbuck)]
    OFF = np.concatenate([[0], np.cumsum(NB)]).astype(np.int64)
    S = int(OFF[-1])
    NBLK = S // P + 1          # +1: self-loop block, filled by plain DMA

    # slot assignment (non-self edges -> window calls)
    ns = np.flatnonzero(~is_self)
    order = ns[np.lexsort((ebuck[ns], etile[ns], ecore[ns]))]
    so_core, so_tile, so_buck = ecore[order], etile[order], ebuck[order]
    # rank within (core,tile,bucket)
    key = (so_core * T + so_tile) * nbuck + so_buck
    uniq, first = np.unique(key, return_index=True)
    rank = np.arange(order.size) - first[np.searchsorted(uniq, key)]
    slot = OFF[so_buck] + rank
    assert (rank < np.array(NB)[so_buck]).all()

    idx16 = np.zeros((M, T, P, S // 16), np.int16)
    # pad slots: local row 0 of own bucket (valid if bucket base < M*NPAD)
    dstloc = np.full((M, T, P, NBLK), 255, np.int16)  # 255 -> all-zero onehot col
    aeh1 = np.full((M, T, P, NBLK, H), ANEG, nbf)
    aeh2 = np.full((M, T, P, NBLK, H), ANEG, nbf)
    nrm3 = np.zeros((M, T, P, NBLK), np.float32)

    c_i, t_i = so_core, so_tile
    lidx0 = gsrc[order] - bases[so_buck]
    assert (lidx0 >= 0).all() and (lidx0 < bucket).all()
    lidx = lidx0.astype(np.int16)
    # self-loop block (last): partition = dst_local, data only for real nodes
    sf = np.flatnonzero(is_self)
    sc, st, sp = ecore[sf], etile[sf], edloc[sf]
    # wrapped idx position within the bucket's call
    off16 = (OFF[so_buck] // 16 + rank // 16).astype(np.int64)
    idx16[c_i, t_i, rank % 16, off16] = lidx
    # dma_gather idxs are read by 8 Q7 cores, each from its own 16-partition
    # replica: partitions 16k..16k+15 must hold identical copies.
    idx16[:, :, 16:32] = idx16[:, :, 0:16]
    for k in range(2, 8):
        idx16[:, :, 16 * k:16 * (k + 1)] = idx16[:, :, 0:16]
    pp, bb2 = (slot % P).astype(np.int64), (slot // P).astype(np.int64)
    dstloc[c_i, t_i, pp, bb2] = edloc[order].astype(np.int16)
    aeh1[c_i, t_i, pp, bb2] = ae1[order].astype(nbf)
    aeh2[c_i, t_i, pp, bb2] = ae2[order].astype(nbf)
    nrm3[c_i, t_i, pp, bb2] = norm[order]
    dstloc[sc, st, sp, NBLK - 1] = sp.astype(np.int16)
    aeh1[sc, st, sp, NBLK - 1] = ae1[sf].astype(nbf)
    aeh2[sc, st, sp, NBLK - 1] = ae2[sf].astype(nbf)
    nrm3[sc, st, sp, NBLK - 1] = norm[sf]
    # host-expanded onehot [T, P(edge), NBLK*P(node)] bf16 per core
    ohh = (dstloc[..., None] == np.arange(P, dtype=np.int16)).astype(nbf)
    ohh = ohh.reshape(M, T, P, NBLK * P)

    # transposed/padded x per core, bf16
    xT = np.zeros((M, P, NPAD), nbf)
    cc = gid // NPAD
    xT[cc, :, gid % NPAD] = x.astype(nbf)

    # set2set masks [P, T] (0 real / SMASK pad)
    smask = np.full((M, P, T), SMASK, np.float32)
    smask[cc, (gid % NPAD) % P, (gid % NPAD) // P] = 0.0

    # ---------------- folded weights (shared across cores) ----------------
    g1, g2 = params['gat1'], params['gat2']
    bn1, bn2, bn3 = params['bn1'], params['bn2'], params['bn3']
    f = lambda a: np.asarray(a, np.float32)

    def bnsc(bn):
        s = f(bn['g']) / np.sqrt(f(bn['v']) + BN_EPS)
        return s, f(bn['b']) - f(bn['m']) * s
    s1, t1 = bnsc(bn1)
    s2, t2 = bnsc(bn2)
    s3, t3 = bnsc(bn3)

    W1b = f(g1['W']).astype(nbf)                          # [128,256]
    att1 = np.broadcast_to(np.concatenate(
        [f(g1['att_src']).reshape(-1), f(g1['att_dst']).reshape(-1)]),
        (P, 2 * HC)).copy()                               # [128,512] f32
    b1b = np.broadcast_to(f(g1['b']), (P, HC)).copy()     # [128,256] f32

    W2f = s1[:, None] * f(g2['W'])                        # [256,256]
    W2fb = W2f.reshape(2, P, HC).astype(nbf)              # chunks [2,128,256]
    b2row = (t1 @ f(g2['W']))[None, :].astype(nbf)        # [1,256]
    att2 = np.broadcast_to(np.concatenate(
        [f(g2['att_src']).reshape(-1), f(g2['att_dst']).reshape(-1)]),
        (P, 2 * HC)).copy()
    s2q = np.broadcast_to(s2 / H, (P, CG)).copy()         # [128,64] f32
    t2q = np.broadcast_to(f(g2['b']) * s2 + t2, (P, CG)).copy()

    W3fb = (f(params['gcn1']['W']) * s3[None, :]).astype(nbf)   # [64,64]
    b3row = (f(params['gcn1']['b']) * s3 + t3)[None, :].astype(nbf)  # [1,64]

    s2s = params['s2s']
    Wih = f(s2s['Wih'])   # [256,128]
    Whh = f(s2s['Whh'])   # [256,64]
    b4 = f(s2s['bih']) + f(s2s['bhh'])                    # [256]
    WihT = np.ascontiguousarray(Wih.reshape(4, CG, P).transpose(2, 0, 1))  # [128(q),4,64(gate)]
    WhhT = np.ascontiguousarray(Whh.reshape(4, CG, CG).transpose(2, 0, 1))  # [64(h),4,64(gate)]
    b4c = np.ascontiguousarray(b4.reshape(4, CG).T)       # [64,4]
    lin1T = f(params['lin1']['W'])                        # [128,64] (lhsT)
    bl1 = f(params['lin1']['b'])[:, None]                 # [64,1]
    lin2T = f(params['lin2']['W'])                        # [64,10]
    bl2 = f(params['lin2']['b'])[:, None]                 # [10,1]

    identF = np.eye(P, dtype=np.float32)
    identB = np.eye(P, dtype=nbf)
    iota = np.broadcast_to(np.arange(P, dtype=np.float32).astype(nbf),
                           (P, P)).copy()
    iotacol = np.arange(P, dtype=np.float32)[:, None].copy()
    ones1b = np.ones((1, P), nbf)
    ones1f = np.ones((1, P), np.float32)
    onescol = np.ones((P, 1), np.float32)

    shared = dict(W1b=W1b, att1=att1, b1b=b1b, W2fb=W2fb, b2row=b2row,
                  att2=att2, s2q=s2q, t2q=t2q, W3fb=W3fb, b3row=b3row,
                  WihT=WihT, WhhT=WhhT, b4c=b4c, lin1T=lin1T, bl1=bl1,
                  lin2T=lin2T, bl2=bl2, identF=identF, identB=identB,
                  iota=iota, iotacol=iotacol, ones1b=ones1b, ones1f=ones1f,
                  onescol=onescol)
    percore = [dict(xT=np.ascontiguousarray(xT[c]),
                    idx16=np.ascontiguousarray(idx16[c]),
                    ohh=np.ascontiguousarray(ohh[c]),
                    aeh1=np.ascontiguousarray(aeh1[c]),
                    aeh2=np.ascontiguousarray(aeh2[c]),
                    nrm3=np.ascontiguousarray(nrm3[c]),
                    smask=np.ascontiguousarray(smask[c]))
               for c in range(M)]
    meta = dict(T=T, TG=TG, GPC=GPC, NPAD=NPAD, S=S, NBLK=NBLK, NB=NB,
                OFF=[int(o) for o in OFF], bucket=bucket, nbuck=nbuck, B=B,
                BASE=[int(b_) for b_ in bases])
    return meta, shared, percore


# ---------------------------------------------------------------- program
def build_program(meta, shared, single=False):
    T, TG, GPC = meta['T'], meta['TG'], meta['GPC']
    NPAD, S, NBLK = meta['NPAD'], meta['S'], meta['NBLK']
    NB, OFF, bucket, nbuck = meta['NB'], meta['OFF'], meta['bucket'], meta['nbuck']

    nc = bacc.Bacc("TRN2", target_bir_lowering=False, debug=False,
                   num_devices=1 if single else M, num_swdge_queues=4)
    D = {}

    def din(name, arr, dt):
        D[name] = nc.dram_tensor(name, list(arr.shape), dt,
                                 kind="ExternalInput").ap()
        return D[name]

    # per-core inputs
    din('xT', np.zeros((P, NPAD)), bf16)
    din('idx16', np.zeros((T, P, S // 16)), i16)
    din('ohh', np.zeros((T, P, NBLK * P)), bf16)
    din('aeh1', np.zeros((T, P, NBLK, H)), bf16)
    din('aeh2', np.zeros((T, P, NBLK, H)), bf16)
    din('nrm3', np.zeros((T, P, NBLK)), f32)
    din('smask', np.zeros((P, T)), f32)
    # shared weights
    for k, v in shared.items():
        din(k, v, bf16 if v.dtype == nbf else f32)

    # internal DRAM
    R1s = nc.dram_tensor("R1s", [NPAD, 384], bf16).ap()
    R2s = nc.dram_tensor("R2s", [NPAD, 384], bf16).ap()
    R3s = nc.dram_tensor("R3s", [NPAD, CG], f32).ap()
    R1f = nc.dram_tensor("R1f", [M * NPAD, 384], bf16, addr_space="Shared").ap()
    R2f = nc.dram_tensor("R2f", [M * NPAD, 384], bf16, addr_space="Shared").ap()
    R3f = nc.dram_tensor("R3f", [M * NPAD, CG], f32, addr_space="Shared").ap()
    yout = nc.dram_tensor("y", [GPC, COUT], f32, kind="ExternalOutput").ap()

    with tile.TileContext(nc) as tc, ExitStack() as ctx:
        _emit(ctx, tc, nc, D, R1s, R2s, R3s, R1f, R2f, R3f, yout, meta,
              single=single)
    return nc


def _emit(ctx, tc, nc, D, R1s, R2s, R3s, R1f, R2f, R3f, yout, meta,
          single=False):
    def allgather(src, dstf):
        if single:
            # cost-model stand-in: local copy of this core's shard
            nc.gpsimd.dma_start(dstf[0:src.shape[0], :], src[:])
        else:
            nc.gpsimd.collective_compute(
                "AllGather", mybir.AluOpType.bypass,
                replica_groups=[list(range(M))],
                ins=[src[:].opt()], outs=[dstf[:].opt()])
    T, TG, GPC = meta['T'], meta['TG'], meta['GPC']
    NPAD, S, NBLK = meta['NPAD'], meta['S'], meta['NBLK']
    NB, OFF, bucket, nbuck = meta['NB'], meta['OFF'], meta['bucket'], meta['nbuck']
    BASE = meta['BASE']
    AG = mybir.AluOpType

    cst = ctx.enter_context(tc.tile_pool(name="cst", bufs=1))
    sb = ctx.enter_context(tc.tile_pool(name="sb", bufs=2))
    res = ctx.enter_context(tc.tile_pool(name="res", bufs=1))

    nbreg = [nc.gpsimd.to_reg(NB[bb]) for bb in range(nbuck)]

    def cload(name, dt=None):
        t = cst.tile(list(D[name].shape), dt or D[name].dtype, tag=name)
        nc.sync.dma_start(t[:], D[name][:])
        return t

    W1b = cload('W1b'); att1 = cload('att1'); b1b = cload('b1b')
    W2fb = cst.tile([P, 2, HC], bf16, tag="W2fb")
    nc.sync.dma_start(W2fb[:], D['W2fb'][:].rearrange("k p q -> p k q"))
    b2row = cload('b2row'); att2 = cload('att2')
    s2q = cload('s2q'); t2q = cload('t2q')
    W3fb = cload('W3fb'); b3row = cload('b3row')
    identF = cload('identF'); identB = cload('identB')
    iota = cload('iota'); iotacol = cload('iotacol')
    ones1b = cload('ones1b'); ones1f = cload('ones1f'); onescol = cload('onescol')
    smask = cload('smask')

    # core-resident state
    adst1 = res.tile([P, T, H], bf16)
    adst2 = res.tile([P, T, H], bf16)
    x2all = res.tile([P, T, CG], f32)
    x3all = res.tile([P, T, CG], bf16)

    # ---------------- phase 0: R1 records ----------------
    ps0 = tc.alloc_tile_pool(name="ps0", bufs=2, space="PSUM")
    ps = ps0
    for t in range(T):
        xTt = sb.tile([P, P], bf16, tag="xTt")
        nc.sync.dma_start(xTt[:], D['xT'][:, bass.ts(t, P)])
        xs = ps.tile([P, HC], f32, space="PSUM", tag="xs")
        nc.tensor.matmul(xs[:], lhsT=xTt[:], rhs=W1b[:], start=True, stop=True)
        rt = sb.tile([P, 384], bf16, tag="rt")
        nc.vector.memset(rt[:, 264:384], 0.0)
        nc.scalar.activation(rt[:, 0:HC], xs[:], mybir.ActivationFunctionType.Copy)
        tmp = sb.tile([P, 2 * HC], f32, tag="attmp")
        nc.vector.tensor_tensor(
            tmp[:].rearrange("p (u q) -> p u q", u=2),
            xs[:].unsqueeze(1).to_broadcast([P, 2, HC]),
            att1[:].rearrange("p (u q) -> p u q", u=2), op=AG.mult)
        asd = sb.tile([P, 2, H], f32, tag="asd")
        nc.vector.tensor_reduce(
            asd[:], tmp[:].rearrange("p (u h c) -> p u h c", u=2, h=H),
            axis=mybir.AxisListType.X, op=AG.add)
        nc.vector.tensor_copy(rt[:, 256:264].bitcast(f32), asd[:, 0, :])
        nc.vector.tensor_copy(adst1[:, t, :], asd[:, 1, :])
        nc.sync.dma_start(R1s[bass.ts(t, P), :], rt[:])

    ps0.release()
    allgather(R1s, R1f)

    # ---------------- GAT layer helper ----------------
    def gat_tile(t, Rf, Rs, aeh_name, adst, layer):
        idx = sb.tile([P, S // 16], i16, tag="idx")
        nc.sync.dma_start(idx[:], D['idx16'][t])
        oh = sb.tile([P, NBLK, P], bf16, tag="oh")
        nc.sync.dma_start(oh[:], D['ohh'][t])
        ae = sb.tile([P, NBLK, H], bf16, tag="ae")
        nc.sync.dma_start(ae[:], D[aeh_name][t])
        G = sb.tile([P, NBLK, 384], bf16, tag="G")
        nc.sync.dma_start(G[:, NBLK - 1, :], Rs[bass.ts(t, P), :])
        for bb in range(nbuck):
            nbb = NB[bb]
            nc.gpsimd.dma_gather(
                G[:, OFF[bb] // P:(OFF[bb] + nbb) // P, :],
                Rf[BASE[bb]:BASE[bb] + bucket, :],
                idx[:, OFF[bb] // 16:(OFF[bb] + nbb) // 16],
                nbb, nbreg[bb], 384, queue_num=bb % 4)
        ohT = sb.tile([P, NBLK, P], bf16, tag="ohT")
        for q in range(cdiv(NBLK, 4)):
            k = min(4, NBLK - 4 * q)
            ohTp = ps.tile([P, 4 * P], bf16, space="PSUM", tag="ohTp")
            for j in range(k):
                nc.tensor.transpose(ohTp[:, bass.ts(j, P)],
                                    oh[:, 4 * q + j, :], identB[:])
            nc.scalar.activation(
                ohT[:, 4 * q:4 * q + k, :].rearrange("p a b -> p (a b)"),
                ohTp[:, 0:k * P], mybir.ActivationFunctionType.Copy)
        aexp = ps.tile([P, NBLK * H], f32, space="PSUM", tag="aexp")
        for bb in range(NBLK):
            nc.tensor.matmul(aexp[:, bass.ts(bb, H)], lhsT=ohT[:, bb, :],
                             rhs=adst[:, t, :], start=True, stop=True)
        a = sb.tile([P, NBLK, H], f32, tag="a")
        nc.vector.tensor_add(a[:], G[:, :, 256:264].bitcast(f32),
                             aexp[:].rearrange("p (b h) -> p b h", h=H))
        nc.vector.tensor_add(a[:], a[:], ae[:])
        nc.vector.scalar_tensor_tensor(a[:], in0=a[:], scalar=0.2, in1=a[:],
                                       op0=AG.mult, op1=AG.max)
        exe = sb.tile([P, NBLK, H, CH], bf16, tag="exe")
        w = sb.tile([P, NBLK, HC], bf16, tag="w")
        nc.scalar.activation(exe[:], a[:].unsqueeze(3).to_broadcast(
            [P, NBLK, H, CH]), mybir.ActivationFunctionType.Exp)
        nc.vector.tensor_tensor(w[:], G[:, :, 0:256], exe[:].rearrange(
            "p b h c -> p (b h c)").rearrange("p (b q) -> p b q", b=NBLK),
            op=AG.mult)
        agg = ps.tile([P, HC], f32, space="PSUM", tag="agg")
        den = ps.tile([P, H], f32, space="PSUM", tag="aexp")
        for bb in range(NBLK):
            nc.tensor.matmul(agg[:], lhsT=oh[:, bb, :], rhs=w[:, bb, :],
                             start=(bb == 0), stop=(bb == NBLK - 1))
            nc.tensor.matmul(den[:], lhsT=oh[:, bb, :], rhs=exe[:, bb, :, 0],
                             start=(bb == 0), stop=(bb == NBLK - 1))
        r4 = sb.tile([P, H], f32, tag="r4")
        nc.vector.tensor_scalar_add(r4[:], den[:], 1e-16)
        nc.vector.reciprocal(r4[:], r4[:])
        return agg, r4

    # ---------------- phase 1: GAT1 + R2 ----------------
    ps1 = tc.alloc_tile_pool(name="ps1", bufs=2, space="PSUM")
    ps = ps1
    for t in range(T):
        agg, r4 = gat_tile(t, R1f, R1s, 'aeh1', adst1, 1)
        x1 = sb.tile([P, HC], f32, tag="x1")
        nc.vector.tensor_tensor(
            x1[:].rearrange("p (h c) -> p h c", h=H),
            agg[:].rearrange("p (h c) -> p h c", h=H),
            r4[:].unsqueeze(2).to_broadcast([P, H, CH]), op=AG.mult)
        nc.vector.tensor_add(x1[:], x1[:], b1b[:])
        # R2 tail: xs2 = bn1(x1) @ W2 (+row bias)
        x1T = sb.tile([P, 2, P], bf16, tag="x1T")
        for k in range(2):
            tp = ps.tile([P, P], f32, space="PSUM", tag="mmA")
            nc.tensor.transpose(tp[:], x1[:, bass.ts(k, P)], identF[:])
            nc.scalar.activation(x1T[:, k, :], tp[:],
                                 mybir.ActivationFunctionType.Copy)
        xs2 = ps.tile([P, HC], f32, space="PSUM", tag="mmA")
        nc.tensor.matmul(xs2[:], lhsT=x1T[:, 0, :], rhs=W2fb[:, 0, :], start=True,
                         stop=False)
        nc.tensor.matmul(xs2[:], lhsT=x1T[:, 1, :], rhs=W2fb[:, 1, :], start=False,
                         stop=False)
        nc.tensor.matmul(xs2[:], lhsT=ones1b[:], rhs=b2row[:], start=False,
                         stop=True)
        rt2 = sb.tile([P, 384], bf16, tag="rt")
        nc.vector.memset(rt2[:, 264:384], 0.0)
        nc.scalar.activation(rt2[:, 0:HC], xs2[:],
                             mybir.ActivationFunctionType.Copy)
        tmp2 = sb.tile([P, 2 * HC], f32, tag="attmp")
        nc.vector.tensor_tensor(
            tmp2[:].rearrange("p (u q) -> p u q", u=2),
            xs2[:].unsqueeze(1).to_broadcast([P, 2, HC]),
            att2[:].rearrange("p (u q) -> p u q", u=2), op=AG.mult)
        asd2 = sb.tile([P, 2, H], f32, tag="asd")
        nc.vector.tensor_reduce(
            asd2[:], tmp2[:].rearrange("p (u h c) -> p u h c", u=2, h=H),
            axis=mybir.AxisListType.X, op=AG.add)
        nc.vector.tensor_copy(rt2[:, 256:264].bitcast(f32), asd2[:, 0, :])
        nc.vector.tensor_copy(adst2[:, t, :], asd2[:, 1, :])
        nc.sync.dma_start(R2s[bass.ts(t, P), :], rt2[:])

    ps1.release()
    allgather(R2s, R2f)

    # ---------------- phase 2: GAT2 + x2 + R3 ----------------
    ps2 = tc.alloc_tile_pool(name="ps2", bufs=2, space="PSUM")
    ps = ps2
    for t in range(T):
        agg, r4 = gat_tile(t, R2f, R2s, 'aeh2', adst2, 2)
        xq = sb.tile([P, HC], f32, tag="xq")
        nc.vector.tensor_tensor(
            xq[:].rearrange("p (h c) -> p h c", h=H),
            agg[:].rearrange("p (h c) -> p h c", h=H),
            r4[:].unsqueeze(2).to_broadcast([P, H, CH]), op=AG.mult)
        x2p = sb.tile([P, CG], f32, tag="x2p")
        nc.vector.tensor_reduce(x2p[:], xq[:].rearrange("p (h c) -> p c h", h=H),
                                axis=mybir.AxisListType.X, op=AG.add)
        nc.vector.tensor_mul(x2p[:], x2p[:], s2q[:])
        nc.vector.tensor_add(x2all[:, t, :], x2p[:], t2q[:])
        # R3 tail: xw = x2 @ W3f
        tp2 = ps.tile([CG, P], f32, space="PSUM", tag="mmA")
        nc.tensor.transpose(tp2[:], x2all[:, t, :], identF[:])
        x2T = sb.tile([CG, P], bf16, tag="x2T")
        nc.scalar.activation(x2T[:], tp2[:], mybir.ActivationFunctionType.Copy)
        xw = ps.tile([P, CG], f32, space="PSUM", tag="mmA")
        nc.tensor.matmul(xw[:], lhsT=x2T[:], rhs=W3fb[:], start=True, stop=True)
        rt3 = sb.tile([P, CG], f32, tag="rt3")
        nc.vector.tensor_copy(rt3[:], xw[:])
        nc.sync.dma_start(R3s[bass.ts(t, P), :], rt3[:])

    ps2.release()
    allgather(R3s, R3f)

    # ---------------- phase 3: GCN + x3 ----------------
    ps3 = tc.alloc_tile_pool(name="ps3", bufs=2, space="PSUM")
    ps = ps3
    for t in range(T):
        idx = sb.tile([P, S // 16], i16, tag="idx")
        nc.sync.dma_start(idx[:], D['idx16'][t])
        oh = sb.tile([P, NBLK, P], bf16, tag="oh")
        nc.sync.dma_start(oh[:], D['ohh'][t])
        nr = sb.tile([P, NBLK], f32, tag="nr")
        nc.sync.dma_start(nr[:], D['nrm3'][t])
        G3 = sb.tile([P, NBLK, CG], f32, tag="G")
        nc.sync.dma_start(G3[:, NBLK - 1, :], R3s[bass.ts(t, P), :])
        for bb in range(nbuck):
            nbb = NB[bb]
            nc.gpsimd.dma_gather(
                G3[:, OFF[bb] // P:(OFF[bb] + nbb) // P, :],
                R3f[BASE[bb]:BASE[bb] + bucket, :],
                idx[:, OFF[bb] // 16:(OFF[bb] + nbb) // 16],
                nbb, nbreg[bb], CG, queue_num=bb % 4)
        w3 = sb.tile([P, NBLK, CG], bf16, tag="w")
        nc.vector.tensor_tensor(
            w3[:], G3[:], nr[:].unsqueeze(2).to_broadcast([P, NBLK, CG]),
            op=AG.mult)
        ag3 = ps.tile([P, CG], f32, space="PSUM", tag="ag3")
        for bb in range(NBLK):
            nc.tensor.matmul(ag3[:], lhsT=oh[:, bb, :], rhs=w3[:, bb, :],
                             start=(bb == 0), stop=False)
        nc.tensor.matmul(ag3[:], lhsT=ones1b[:], rhs=b3row[:], start=False,
                         stop=True)
        v3 = sb.tile([P, CG], f32, tag="v3")
        nc.vector.tensor_add(v3[:], ag3[:], x2all[:, t, :])
        nc.vector.scalar_tensor_tensor(x3all[:, t, :], in0=v3[:], scalar=0.01,
                                       in1=v3[:], op0=AG.mult, op1=AG.max)

    # ---------------- phase 4: Set2Set + MLP ----------------
    ps3.release()
    ps = tc.alloc_tile_pool(name="ps4", bufs=1, space="PSUM")
    qsT = res.tile([P, GPC], f32)       # [h ; r]
    cT = res.tile([CG, GPC], f32)
    nc.vector.memset(qsT[:], 0.0)
    nc.vector.memset(cT[:], 0.0)
    SIG = mybir.ActivationFunctionType.Sigmoid
    TANH = mybir.ActivationFunctionType.Tanh
    WihT = cst.tile([P, 4, CG], f32, tag="WihT")
    nc.sync.dma_start(WihT[:], D['WihT'][:])
    WhhT = cst.tile([CG, 4, CG], f32, tag="WhhT")
    nc.sync.dma_start(WhhT[:], D['WhhT'][:])
    b4c = cload('b4c'); lin1T = cload('lin1T'); bl1 = cload('bl1')
    lin2T = cload('lin2T'); bl2 = cload('bl2')

    for step in range(LSTM_STEPS):
        # gates: 4x [64,GPC] all on partitions 0:64 (i,f,g,o)
        g4 = ps.tile([CG, 4 * GPC], f32, space="PSUM", tag="g4")
        for gi in range(4):
            nc.tensor.matmul(g4[:, bass.ts(gi, GPC)], lhsT=WihT[:, gi, :],
                             rhs=qsT[:], start=True, stop=False)
            nc.tensor.matmul(g4[:, bass.ts(gi, GPC)], lhsT=WhhT[:, gi, :],
                             rhs=qsT[0:CG, :], start=False, stop=True)
        gg = sb.tile([CG, 4 * GPC], f32, tag="gg")
        for gi, fn in enumerate([SIG, SIG, TANH, SIG]):
            nc.scalar.activation(gg[:, bass.ts(gi, GPC)],
                                 g4[:, bass.ts(gi, GPC)], fn,
                                 bias=b4c[:, gi:gi + 1])
        iT, fT = gg[:, 0:GPC], gg[:, GPC:2 * GPC]
        gT, oT = gg[:, 2 * GPC:3 * GPC], gg[:, 3 * GPC:4 * GPC]
        t1_ = sb.tile([CG, GPC], f32, tag="t1_")
        nc.vector.tensor_mul(cT[:], fT, cT[:])
        nc.vector.tensor_mul(t1_[:], iT, gT)
        nc.vector.tensor_add(cT[:], cT[:], t1_[:])
        th = sb.tile([CG, GPC], f32, tag="th")
        nc.scalar.activation(th[:], cT[:], TANH)
        nc.vector.tensor_mul(qsT[0:CG, :], oT, th[:])
        # h_row [1, GPC*CG] via per-graph identity matmuls
        hr = ps.tile([1, GPC * CG], f32, space="PSUM", tag="hrsig")
        for g in range(GPC):
            nc.tensor.matmul(hr[:, bass.ts(g, CG)],
                             lhsT=qsT[0:CG, g:g + 1],
                             rhs=identF[0:CG, 0:CG], start=True, stop=True)
        hrs = sb.tile([1, GPC * CG], f32, tag="hrs")
        nc.vector.tensor_copy(hrs[:], hr[:])
        hb = ps.tile([P, GPC * CG], f32, space="PSUM", tag="hbsrb")
        nc.tensor.matmul(hb[:], lhsT=ones1f[:], rhs=hrs[:], start=True,
                         stop=True)
        hbb = sb.tile([P, GPC * CG], bf16, tag="hbb")
        nc.scalar.activation(hbb[:], hb[:], mybir.ActivationFunctionType.Copy)
        # e = sum_f x3*h
        tmpe = sb.tile([P, T * CG], bf16, tag="G")
        nc.vector.tensor_tensor(
            tmpe[:].rearrange("p (g u c) -> p g u c", g=GPC, u=TG),
            x3all[:].rearrange("p (g u) c -> p g u c", g=GPC),
            hbb[:].rearrange("p (g c) -> p g c", g=GPC).unsqueeze(2)
                  .to_broadcast([P, GPC, TG, CG]), op=AG.mult)
        e = sb.tile([P, T], f32, tag="e")
        nc.vector.tensor_reduce(e[:], tmpe[:].rearrange("p (t c) -> p t c", c=CG),
                                axis=mybir.AxisListType.X, op=AG.add)
        nc.vector.tensor_add(e[:], e[:], smask[:])
        ex = sb.tile([P, T], f32, tag="ex")
        nc.scalar.activation(ex[:], e[:], mybir.ActivationFunctionType.Exp)
        sig = ps.tile([1, T], f32, space="PSUM", tag="hrsig")
        nc.tensor.matmul(sig[:], lhsT=onescol[:], rhs=ex[:], start=True,
                         stop=True)
        sg = sb.tile([1, GPC], f32, tag="sg")
        nc.vector.tensor_reduce(sg[:], sig[:].rearrange("p (g u) -> p g u", g=GPC),
                                axis=mybir.AxisListType.X, op=AG.add)
        nc.vector.tensor_scalar_add(sg[:], sg[:], 1e-16)
        nc.vector.reciprocal(sg[:], sg[:])
        srb = ps.tile([P, GPC], f32, space="PSUM", tag="hbsrb")
        nc.tensor.matmul(srb[:], lhsT=ones1f[:], rhs=sg[:], start=True,
                         stop=True)
        exn = sb.tile([P, T], bf16, tag="exn")
        nc.vector.tensor_tensor(
            exn[:].rearrange("p (g u) -> p g u", g=GPC),
            ex[:].rearrange("p (g u) -> p g u", g=GPC),
            srb[:].unsqueeze(2).to_broadcast([P, GPC, TG]), op=AG.mult)
        rt_ = ps.tile([P, GPC], f32, space="PSUM", tag="rt_")
        for g in range(GPC):
            for u in range(TG):
                nc.tensor.matmul(rt_[CG:P, g:g + 1],
                                 lhsT=x3all[:, g * TG + u, :],
                                 rhs=exn[:, g * TG + u:g * TG + u + 1],
                                 start=(u == 0), stop=(u == TG - 1))
        nc.vector.tensor_copy(qsT[CG:P, :], rt_[CG:P, :])

    y1p = ps.tile([CG, GPC], f32, space="PSUM", tag="rtmm")
    nc.tensor.matmul(y1p[:], lhsT=lin1T[:], rhs=qsT[:], start=True, stop=True)
    y1 = sb.tile([CG, GPC], f32, tag="y1")
    nc.vector.tensor_scalar_add(y1[:], y1p[:], bl1[:])
    nc.vector.scalar_tensor_tensor(y1[:], in0=y1[:], scalar=0.01, in1=y1[:],
                                   op0=AG.mult, op1=AG.max)
    y2p = ps.tile([COUT, GPC], f32, space="PSUM", tag="rtmm")
    nc.tensor.matmul(y2p[:], lhsT=lin2T[:], rhs=y1[:], start=True, stop=True)
    y2 = sb.tile([COUT, GPC], f32, tag="y2")
    nc.vector.tensor_scalar_add(y2[:], y2p[:], bl2[:])
    yTp = ps.tile([GPC, COUT], f32, space="PSUM", tag="rtmm")
    nc.tensor.transpose(yTp[:], y2[:], identF[0:COUT, 0:COUT])
    yf = sb.tile([GPC, COUT], f32, tag="yf")
    nc.vector.tensor_copy(yf[:], yTp[:])
    nc.sync.dma_start(yout[:], yf[:])
    ps.release()


# ---------------------------------------------------------------- runner
def prepare(x, edge_index, batch_idx, edge_attr, params, bucket=32768,
            nwin=None):
    meta, shared, percore = preprocess(x, edge_index, batch_idx, edge_attr,
                                       params, bucket=bucket, nwin=nwin)
    nc = build_program(meta, shared)
    nc.finalize()
    in_maps = []
    for c in range(M):
        m = dict(percore[c])
        m.update(shared)
        in_maps.append({k: np.ascontiguousarray(v) for k, v in m.items()})
    return nc, in_maps, meta


def execute(nc, in_maps, meta, use_sim=False, trace=False):
    if use_sim:
        from concourse.bass_interp import MultiCoreSim
        sim = MultiCoreSim(nc, num_cores=M, trace=False,
                   require_finite=False, require_nnan=False)
        for c, core in sim.cores.items():
            for k, v in in_maps[c].items():
                core.tensor(k)[:] = v
        sim.simulate()
        ys = [np.array(sim.cores[c].tensor("y")) for c in range(M)]
    else:
        from concourse.bass_utils import run_bass_kernel_spmd
        r = run_bass_kernel_spmd(nc, in_maps, core_ids=list(range(M)),
                                 trace=trace)
        ys = [r.results[c]["y"] for c in range(M)]
        execute.last_results = r
    GPC = meta['GPC']
    out = np.zeros((meta['B'], COUT), np.float32)
    for c in range(M):
        out[c * GPC:(c + 1) * GPC] = ys[c]
    return out


def run(x, edge_index, batch_idx, edge_attr, params, bucket=32768,
        use_sim=False, trace=False):
    nc, in_maps, meta = prepare(x, edge_index, batch_idx, edge_attr, params,
                                bucket=bucket)
    return execute(nc, in_maps, meta, use_sim=use_sim, trace=trace)


# ---------------------------------------------------------------- entry point
def kernel(x, edge_index, batch_idx, edge_attr, params):
    """Full-input kernel: shards across 8 NeuronCores internally."""
    params = {k: ({kk: np.asarray(vv) for kk, vv in v.items()}
                  if isinstance(v, dict) else np.asarray(v))
              for k, v in params.items()}
    nc, in_maps, meta = prepare(np.asarray(x), np.asarray(edge_index),
                                np.asarray(batch_idx), np.asarray(edge_attr),
                                params)
    return execute(nc, in_maps, meta)
